# revision 31
# baseline (speedup 1.0000x reference)
"""Multi-head attention (B=2, S=2048, D=768, H=12) on 8 trn2 NeuronCores.

Sharding: 24 (batch, head) pairs split 3-heads-per-core (core c -> batch
c//4, heads 3*(c%4)..+2).  Each core: Q/K/V projections for its heads,
attention, and a partial output projection against its 192-col slice of
w_o.  Host sums the per-batch partials (f32 from f16 partials).

v2 design (PE-bound; everything f16 on device, f32 only in PSUM):
 - host pre-casts inputs+weights to f16 and pre-transposes, halving input
   DMA and removing all on-device f32->f16 cast traffic
 - projections for heads 0,1 run as one M=128 matmul group (pair tiles
   qT01/kT01/vT01 [128,S]); head 2 separate M=64 tiles
 - out-projection contracts K=128 (pair) + K=64 (head 2) per out-chunk
 - scores computed transposed (S^T = K Q^T) per 128-key chunk into
   [128,1024] PSUM slabs; exp on the scalar engine with the 1/sqrt(dk)
   scale folded into the activation scale
 - AV uses vaug [128,65] (ones column -> denominators for free)
 - softmax norm: evict xp, DVE reciprocal of the den row, gpsimd
   partition_broadcast, DVE multiply (cross-partition write packs heads
   0,1 into xt01 [128,S])
 - global 96-slot weave: per slot [exp(t) | scores(t+1) | AV(t-2) |
   fillers]; fillers carry remaining projections, vaug builds and the
   early out-projection so the PE never idles
"""

import sys

sys.path.insert(0, "/opt/trn_rl_repo")

import numpy as np

B, S, D = 2, 2048, 768
H, DK = 12, 64
HPC = 3          # heads per core
DP = HPC * DK    # 192 output dims per core per kind
NCORES = 8
DCH = D // 128   # 6 d-chunks
KC = S // 128    # 16 key chunks
QB = 512         # matmul N block
EB = 1024        # exp slab width
NEB = S // EB    # 2
SCALE = 1.0 / 8.0

_compiled = {}
import os as _os
_DBG = bool(_os.environ.get("KERNEL_DBG"))


def _build():
    import concourse.bass as bass
    import concourse.mybir as mybir
    import concourse.tile as tile
    from concourse import bacc
    from concourse.masks import make_identity

    f32 = mybir.dt.float32
    f16 = mybir.dt.float16
    MULT = mybir.AluOpType.mult
    EXP = mybir.ActivationFunctionType.Exp

    nc = bacc.Bacc("TRN2", target_bir_lowering=False, debug=False)

    xqt = nc.dram_tensor("xqt", [D, S], f16, kind="ExternalInput")
    xkt = nc.dram_tensor("xkt", [D, S], f16, kind="ExternalInput")
    xvt = nc.dram_tensor("xvt", [D, S], f16, kind="ExternalInput")
    # weights pre-packed [128, DCH*DP] partition-major so the DMA moves one
    # contiguous 2.3KB line per partition (the [D, DP] layout degenerates to
    # 384B descriptors and takes ~12us to land)
    wqt = nc.dram_tensor("wqt", [128, DCH * DP], f16, kind="ExternalInput")
    wkt = nc.dram_tensor("wkt", [128, DCH * DP], f16, kind="ExternalInput")
    wvt = nc.dram_tensor("wvt", [128, DCH * DP], f16, kind="ExternalInput")
    wot = nc.dram_tensor("wot", [DP, D], f16, kind="ExternalInput")
    outt = nc.dram_tensor("outt", [D, S], f16, kind="ExternalOutput")
    if _DBG:
        dbg_qT = nc.dram_tensor("dbg_qT", [128, S], f16, kind="ExternalOutput")
        dbg_kT = nc.dram_tensor("dbg_kT", [128, S], f16, kind="ExternalOutput")
        dbg_va = nc.dram_tensor("dbg_va", [128, KC * 65], f16, kind="ExternalOutput")
        dbg_pt = nc.dram_tensor("dbg_pt", [128, EB], f16, kind="ExternalOutput")
        dbg_xt = nc.dram_tensor("dbg_xt", [128, S], f16, kind="ExternalOutput")
        dbg_x2 = nc.dram_tensor("dbg_x2", [64, S], f16, kind="ExternalOutput")
        dbg_xc = nc.dram_tensor("dbg_xc", [65, QB], f32, kind="ExternalOutput")
        dbg_rd = nc.dram_tensor("dbg_rd", [1, QB], f32, kind="ExternalOutput")
        dbg_rdb = nc.dram_tensor("dbg_rdb", [64, QB], f32, kind="ExternalOutput")

    with tile.TileContext(nc) as tc:
        with (
            tc.tile_pool(name="res", bufs=1) as res,
            tc.tile_pool(name="pt", bufs=16) as pt_pool,
            tc.tile_pool(name="xc", bufs=4) as xc_pool,
            tc.tile_pool(name="ob", bufs=2) as ob_pool,
            tc.tile_pool(name="psS", bufs=2, space="PSUM") as psS,
            tc.tile_pool(name="psX", bufs=2, space="PSUM") as psX,
            tc.tile_pool(name="psP", bufs=2, space="PSUM") as psP,
        ):
            ring_state = [0]

            def ring3():
                ring_state[0] = (ring_state[0] + 1) % 3
                return (nc.sync, nc.scalar, nc.gpsimd)[ring_state[0]]

            ring2_state = [0]

            def ring2():
                # mid-kernel rings: keep the Act sequencer free for exps
                ring2_state[0] ^= 1
                return (nc.sync, nc.gpsimd)[ring2_state[0]]

            # ---- resident tiles ----
            wq_bf = res.tile([128, DCH, DP], f16, tag="wq")
            wk_bf = res.tile([128, DCH, DP], f16, tag="wk")
            wv_bf = res.tile([128, DCH, DP], f16, tag="wv")
            wo_pair = res.tile([128, D], f16, tag="wop")
            wo_h2 = res.tile([64, D], f16, tag="wo2")
            qt_bf = res.tile([128, DCH, S], f16, tag="qt")
            kt_bf = res.tile([128, DCH, S], f16, tag="kt")
            vt_bf = res.tile([128, DCH, S], f16, tag="vt")
            qT01 = res.tile([128, S], f16, tag="qT01")
            kT01 = res.tile([128, S], f16, tag="kT01")
            vT01 = res.tile([128, S], f16, tag="vT01")
            qT2 = res.tile([64, S], f16, tag="qT2")
            kT2 = res.tile([64, S], f16, tag="kT2")
            vT2 = res.tile([64, S], f16, tag="vT2")
            vaug = [
                res.tile([128, KC, 65], f16, tag=f"vaug{h}", name=f"vaug{h}")
                for h in range(HPC)
            ]
            xt01 = res.tile([128, S], f16, tag="xt01")
            xt2 = res.tile([64, S], f16, tag="xt2")
            ident = res.tile([128, 64], f16, tag="ident")

            # ---- prologue DMAs: weights (1 descriptor/partition), then
            # k chunks, q chunks, v in 2-chunk groups ----
            nc.sync.dma_start(wq_bf[:], wqt.rearrange("p (c o) -> p c o", c=DCH))
            nc.scalar.dma_start(wk_bf[:], wkt.rearrange("p (c o) -> p c o", c=DCH))
            nc.gpsimd.dma_start(wv_bf[:], wvt.rearrange("p (c o) -> p c o", c=DCH))
            for i in range(DCH):
                ring3().dma_start(kt_bf[:, i, :], xkt[128 * i : 128 * (i + 1), :])
            for i in range(DCH):
                ring3().dma_start(qt_bf[:, i, :], xqt[128 * i : 128 * (i + 1), :])
            for g in range(3):
                ring3().dma_start(
                    vt_bf[:, 2 * g : 2 * g + 2, :],
                    xvt[256 * g : 256 * (g + 1), :].rearrange(
                        "(c p) s -> p c s", p=128
                    ),
                )
            nc.sync.dma_start(wo_pair[:], wot[0:128, :])
            nc.gpsimd.dma_start(wo_h2[:], wot[128:DP, :])
            make_identity(nc, ident[0:64, :])
            nc.vector.tensor_copy(ident[64:128, :], ident[0:64, :])

            # ---- projection groups ----
            KINDS = {
                "q": (wq_bf, qt_bf, qT01, qT2),
                "k": (wk_bf, kt_bf, kT01, kT2),
                "v": (wv_bf, vt_bf, vT01, vT2),
            }

            def proj_group(kind, mt, j):
                """One M-tile (mt=0: heads01 pair M=128; mt=1: head2 M=64)
                of a 512-col projection block j."""
                wbf, xbf, dpair, dsingle = KINDS[kind]
                if mt == 0:
                    pp = psP.tile([128, QB], f32, tag="pp")
                    wsl = slice(0, 128)
                    dst = dpair[:, QB * j : QB * (j + 1)]
                else:
                    pp = psP.tile([64, QB], f32, tag="pp")
                    wsl = slice(128, DP)
                    dst = dsingle[:, QB * j : QB * (j + 1)]
                for i in range(DCH):
                    nc.tensor.matmul(
                        pp[:],
                        wbf[:, i, wsl],
                        xbf[:, i, QB * j : QB * (j + 1)],
                        start=(i == 0),
                        stop=(i == DCH - 1),
                    )
                if kind == "q":
                    # fold the 1/sqrt(dk) softmax scale into the Q eviction
                    nc.vector.tensor_scalar_mul(dst, pp[:], SCALE)
                else:
                    nc.vector.tensor_copy(dst, pp[:])

            def vaug_unit(h, t4):
                """4 PE transposes of head h's vT -> vaug rows + evict."""
                def emit():
                    if t4 == 0:
                        nc.vector.memset(vaug[h][:, :, 64:65], 1.0)
                    if h == 0:
                        src = lambda t: vT01[0:64, 128 * t : 128 * (t + 1)]
                        idn = ident[0:64, :]
                    elif h == 1:
                        src = lambda t: vT01[64:128, 128 * t : 128 * (t + 1)]
                        idn = ident[64:128, :]
                    else:
                        src = lambda t: vT2[:, 128 * t : 128 * (t + 1)]
                        idn = ident[0:64, :]
                    tp = psP.tile([128, 4, 64], f16, tag="pp")
                    for u in range(4):
                        nc.tensor.transpose(tp[:, u, :], src(4 * t4 + u), idn)
                    nc.vector.tensor_copy(
                        vaug[h][:, 4 * t4 : 4 * t4 + 4, 0:64], tp[:]
                    )
                return emit

            ob_tiles = {}

            def outproj_unit(j, m, tail=False):
                def emit():
                    if m == 0:
                        ob_tiles[j] = ob_pool.tile(
                            [128, DCH, QB], f16, tag="ob", name="ob"
                        )
                    op = psP.tile([128, QB], f32, tag="pp")
                    nc.tensor.matmul(
                        op[:],
                        wo_pair[:, 128 * m : 128 * (m + 1)],
                        xt01[:, QB * j : QB * (j + 1)],
                        start=True,
                        stop=False,
                    )
                    nc.tensor.matmul(
                        op[:],
                        wo_h2[:, 128 * m : 128 * (m + 1)],
                        xt2[:, QB * j : QB * (j + 1)],
                        start=False,
                        stop=True,
                    )
                    if tail and m % 2 == 0:
                        nc.scalar.copy(ob_tiles[j][:, m, :], op[:])
                    else:
                        nc.vector.tensor_copy(ob_tiles[j][:, m, :], op[:])
                    # per-chunk DMA: starts draining while later chunks evict
                    ring2().dma_start(
                        outt.rearrange("(c p) s -> p c s", p=128)[
                            :, m, QB * j : QB * (j + 1)
                        ],
                        ob_tiles[j][:, m, :],
                    )
                return emit

            # ---- slab machinery ----
            slabs = [(0, 0), (1, 0), (2, 0), (0, 1), (1, 1), (2, 1)]  # (h, e)

            def head_rows(h, tensor01, tensor2):
                if h == 0:
                    return tensor01[0:64, :]
                if h == 1:
                    return tensor01[64:128, :]
                return tensor2[:, :]

            sp_tiles = {}
            pt_tiles = {}
            xp_tiles = {}

            def emit_scores(si, t):
                h, e = slabs[si]
                kT = head_rows(h, kT01, kT2)
                qT = head_rows(h, qT01, qT2)
                sp = psS.tile([128, EB], f32, tag="sp")
                sp_tiles[(si, t)] = sp
                for half in range(2):
                    nc.tensor.matmul(
                        sp[:, QB * half : QB * (half + 1)],
                        kT[:, 128 * t : 128 * (t + 1)],
                        qT[:, EB * e + QB * half : EB * e + QB * (half + 1)],
                        start=True,
                        stop=True,
                    )

            def emit_exp(si, t):
                pt = pt_pool.tile([128, EB], f16, tag="pt")
                pt_tiles[(si, t)] = pt
                nc.scalar.activation(pt[:], sp_tiles.pop((si, t))[:], EXP)
                if _DBG and si == 0 and t == 0:
                    nc.sync.dma_start(dbg_pt[:], pt[:])

            def emit_norm_half(si, a, xp):
                """Evict one [65,512] AV accumulator and normalize into xt."""
                h, e = slabs[si]
                cols = slice(EB * e + QB * a, EB * e + QB * (a + 1))
                xca = xc_pool.tile([65, QB], f32, tag="xc")
                nc.vector.tensor_copy(xca[:], xp[:])
                # partition_broadcast reads ABSOLUTE partition 0 on HW, so the
                # den row must first move to a base-0 tile
                d0 = xc_pool.tile([1, QB], f32, tag="rd")
                nc.vector.tensor_copy(d0[:], xca[64:65, :])
                rdenb = xc_pool.tile([64, QB], f32, tag="rdb")
                nc.gpsimd.partition_broadcast(rdenb[:], d0[:])
                nc.vector.reciprocal_approx_fast(rdenb[:], rdenb[:])
                if h == 0:
                    dst = xt01[0:64, cols]
                elif h == 1:
                    dst = xt01[64:128, cols]
                else:
                    dst = xt2[:, cols]
                nc.vector.tensor_tensor(dst, xca[0:64, :], rdenb[:], MULT)
                if _DBG and si == 0 and a == 0:
                    nc.sync.dma_start(dbg_xc[:], xca[:])
                    nc.sync.dma_start(dbg_rdb[:], rdenb[:])
                normed[2 * e + a] += 1

            def emit_av(si, t):
                h, e = slabs[si]
                if si not in xp_tiles:
                    xp_tiles[si] = (
                        psX.tile([65, QB], f32, tag="xp", name="xpa"),
                        psX.tile([65, QB], f32, tag="xp", name="xpb"),
                    )
                xpa, xpb = xp_tiles[si]
                pt = pt_tiles[(si, t)]
                nc.tensor.matmul(
                    xpa[:], vaug[h][:, t, :], pt[:, 0:QB],
                    start=(t == 0), stop=(t == KC - 1),
                )
                nc.tensor.matmul(
                    xpb[:], vaug[h][:, t, :], pt[:, QB:EB],
                    start=(t == 0), stop=(t == KC - 1),
                )
                if t == KC - 1:
                    emit_norm_half(si, 0, xpa)
                    emit_norm_half(si, 1, xpb)
                    del xp_tiles[si]
                    for u in range(KC):
                        del pt_tiles[(si, u)]

            # ---- filler queue (order encodes deadlines) ----
            vaug_done = [0, 0, 0]
            normed = [0, 0, 0, 0]  # heads normed per 512-col block

            def count_vaug(h, t4):
                u = vaug_unit(h, t4)
                def emit():
                    u()
                    vaug_done[h] += 1
                return emit

            fillers = []
            fillers.append(lambda: proj_group("k", 0, 1))
            fillers.append(lambda: proj_group("v", 0, 0))
            fillers.append(lambda: proj_group("k", 0, 2))
            fillers.append(lambda: proj_group("v", 0, 1))
            fillers.append(lambda: proj_group("k", 0, 3))
            fillers.append(count_vaug(0, 0))
            fillers.append(lambda: proj_group("v", 0, 2))
            fillers.append(count_vaug(0, 1))
            fillers.append(lambda: proj_group("v", 0, 3))
            fillers.append(count_vaug(0, 2))
            fillers.append(count_vaug(0, 3))
            for t4 in range(4):
                fillers.append(count_vaug(1, t4))
            # head-2 projections (needed from slab 2 = slot 32)
            for j in range(4):
                fillers.append(lambda j=j: proj_group("k", 1, j))
            fillers.append(lambda: proj_group("q", 1, 0))
            fillers.append(lambda: proj_group("q", 1, 1))
            for j in range(4):
                fillers.append(lambda j=j: proj_group("v", 1, j))
            for t4 in range(4):
                fillers.append(count_vaug(2, t4))
            # q cols 1024-2047 (needed from slab 3 = slot 48)
            fillers.append(lambda: proj_group("q", 0, 2))
            fillers.append(lambda: proj_group("q", 0, 3))
            fillers.append(lambda: proj_group("q", 1, 2))
            fillers.append(lambda: proj_group("q", 1, 3))
            # early out-projection: reserved for slabs 4-5 where the PE
            # otherwise idles at Act pace (scores can't run ahead of exp)
            late_fillers = []
            for j in (0, 1):
                for m in range(DCH):
                    late_fillers.append(
                        (lambda j=j: normed[j] >= HPC, outproj_unit(j, m))
                    )

            # ---- prologue projections: enough for slab 0 ----
            proj_group("k", 0, 0)
            proj_group("q", 0, 0)
            proj_group("q", 0, 1)

            # ---- the 96-slot weave ----
            av_due = []   # (global_slot_emitted, si, t)
            av_ptr = [0]

            def pop_avs(s_now, budget):
                n = 0
                while n < budget and av_ptr[0] < len(av_due):
                    s_e, si, t = av_due[av_ptr[0]]
                    if s_e > s_now - 2:
                        break
                    h, _ = slabs[si]
                    if vaug_done[h] <= t // 4:
                        break
                    emit_av(si, t)
                    av_ptr[0] += 1
                    n += 1
                return n

            emit_scores(0, 0)
            for s in range(96):
                si, t = divmod(s, 16)
                emit_exp(si, t)
                av_due.append((s, si, t))
                if t < KC - 1:
                    emit_scores(si, t + 1)
                elif si < len(slabs) - 1:
                    emit_scores(si + 1, 0)
                npop = pop_avs(s, 5 if s >= 88 else 3)
                nf = 2 if npop == 0 else 1
                for _ in range(nf):
                    if not fillers:
                        break
                    head = fillers[0]
                    if isinstance(head, tuple):
                        gate, fn = head
                        if not gate():
                            break
                        fillers.pop(0)
                        fn()
                    else:
                        fillers.pop(0)()
                if s >= 48 and late_fillers and (not fillers or s % 2 == 0):
                    gate, fn = late_fillers[0]
                    if gate():
                        late_fillers.pop(0)
                        fn()

            # ---- tail: drain AVs/norms, leftover fillers, outproj j2,j3 ----
            while av_ptr[0] < len(av_due):
                pop_avs(10**9, 10**9)
            for u in fillers + late_fillers:
                u[1]() if isinstance(u, tuple) else u()
            for j in (2, 3):
                for m in range(DCH):
                    outproj_unit(j, m, tail=True)()
            if _DBG:
                nc.sync.dma_start(dbg_qT[:], qT01[:])
                nc.sync.dma_start(dbg_kT[:], kT01[:])
                nc.sync.dma_start(
                    dbg_va[:], vaug[0].rearrange("p a b -> p (a b)")
                )
                nc.sync.dma_start(dbg_xt[:], xt01[:])
                nc.sync.dma_start(dbg_x2[:], xt2[:])

    nc.compile()
    return nc


def _get_nc():
    if "nc" not in _compiled:
        _compiled["nc"] = _build()
    return _compiled["nc"]


def _pack_w(w, cols):
    # [128, DCH*DP]: partition p, free = (chunk c, outdim o) of w[cols].T
    wt = w[cols, :].T.astype(np.float16)          # [D, DP]
    return np.ascontiguousarray(
        wt.reshape(DCH, 128, DP).transpose(1, 0, 2).reshape(128, DCH * DP)
    )


def _shard(q, k, v, w_q, w_k, w_v, w_o):
    f16 = np.float16
    in_maps = []
    for c in range(NCORES):
        b, g = divmod(c, NCORES // B)
        cols = slice(DP * g, DP * (g + 1))
        in_maps.append(
            {
                "xqt": np.ascontiguousarray(q[b].T.astype(f16)),
                "xkt": np.ascontiguousarray(k[b].T.astype(f16)),
                "xvt": np.ascontiguousarray(v[b].T.astype(f16)),
                "wqt": _pack_w(w_q, cols),
                "wkt": _pack_w(w_k, cols),
                "wvt": _pack_w(w_v, cols),
                "wot": np.ascontiguousarray(w_o[:, cols].T.astype(f16)),
            }
        )
    return in_maps


def kernel(q, k, v, w_q, w_k, w_v, w_o, _trace=False):
    from concourse.bass_utils import run_bass_kernel_spmd

    q = np.asarray(q, np.float32)
    k = np.asarray(k, np.float32)
    v = np.asarray(v, np.float32)
    w_q = np.asarray(w_q, np.float32)
    w_k = np.asarray(w_k, np.float32)
    w_v = np.asarray(w_v, np.float32)
    w_o = np.asarray(w_o, np.float32)

    nc = _get_nc()
    in_maps = _shard(q, k, v, w_q, w_k, w_v, w_o)
    res = run_bass_kernel_spmd(nc, in_maps, list(range(NCORES)), trace=_trace)
    out = np.zeros((B, S, D), np.float32)
    for c in range(NCORES):
        b = c // (NCORES // B)
        out[b] += res.results[c]["outt"].T.astype(np.float32)
    if _trace:
        return out, res
    return out


# revision 32
# speedup vs baseline: 1.0264x; 1.0264x over previous
"""Multi-head attention (B=2, S=2048, D=768, H=12) on 8 trn2 NeuronCores.

Sharding: 24 (batch, head) pairs split 3-heads-per-core (core c -> batch
c//4, heads 3*(c%4)..+2).  Each core: Q/K/V projections for its heads,
attention, and a partial output projection against its 192-col slice of
w_o.  Host sums the per-batch partials (f32 from f16 partials).

v2 design (PE-bound; everything f16 on device, f32 only in PSUM):
 - host pre-casts inputs+weights to f16 and pre-transposes, halving input
   DMA and removing all on-device f32->f16 cast traffic
 - projections for heads 0,1 run as one M=128 matmul group (pair tiles
   qT01/kT01/vT01 [128,S]); head 2 separate M=64 tiles
 - out-projection contracts K=128 (pair) + K=64 (head 2) per out-chunk
 - scores computed transposed (S^T = K Q^T) per 128-key chunk into
   [128,1024] PSUM slabs; exp on the scalar engine with the 1/sqrt(dk)
   scale folded into the activation scale
 - AV uses vaug [128,65] (ones column -> denominators for free)
 - softmax norm: evict xp, DVE reciprocal of the den row, gpsimd
   partition_broadcast, DVE multiply (cross-partition write packs heads
   0,1 into xt01 [128,S])
 - global 96-slot weave: per slot [exp(t) | scores(t+1) | AV(t-2) |
   fillers]; fillers carry remaining projections, vaug builds and the
   early out-projection so the PE never idles
"""

import sys

sys.path.insert(0, "/opt/trn_rl_repo")

import numpy as np

B, S, D = 2, 2048, 768
H, DK = 12, 64
HPC = 3          # heads per core
DP = HPC * DK    # 192 output dims per core per kind
NCORES = 8
DCH = D // 128   # 6 d-chunks
KC = S // 128    # 16 key chunks
QB = 512         # matmul N block
EB = 1024        # exp slab width
NEB = S // EB    # 2
SCALE = 1.0 / 8.0

_compiled = {}
import os as _os
_DBG = bool(_os.environ.get("KERNEL_DBG"))


def _build():
    import concourse.bass as bass
    import concourse.mybir as mybir
    import concourse.tile as tile
    from concourse import bacc
    from concourse.masks import make_identity

    f32 = mybir.dt.float32
    f16 = mybir.dt.float16
    MULT = mybir.AluOpType.mult
    EXP = mybir.ActivationFunctionType.Exp

    nc = bacc.Bacc("TRN2", target_bir_lowering=False, debug=False)

    xqt = nc.dram_tensor("xqt", [D, S], f16, kind="ExternalInput")
    xkt = nc.dram_tensor("xkt", [D, S], f16, kind="ExternalInput")
    xvt = nc.dram_tensor("xvt", [D, S], f16, kind="ExternalInput")
    # weights pre-packed [128, DCH*DP] partition-major so the DMA moves one
    # contiguous 2.3KB line per partition (the [D, DP] layout degenerates to
    # 384B descriptors and takes ~12us to land)
    wqt = nc.dram_tensor("wqt", [128, DCH * DP], f16, kind="ExternalInput")
    wkt = nc.dram_tensor("wkt", [128, DCH * DP], f16, kind="ExternalInput")
    wvt = nc.dram_tensor("wvt", [128, DCH * DP], f16, kind="ExternalInput")
    wot = nc.dram_tensor("wot", [DP, D], f16, kind="ExternalInput")
    outt = nc.dram_tensor("outt", [D, S], f16, kind="ExternalOutput")
    if _DBG:
        dbg_qT = nc.dram_tensor("dbg_qT", [128, S], f16, kind="ExternalOutput")
        dbg_kT = nc.dram_tensor("dbg_kT", [128, S], f16, kind="ExternalOutput")
        dbg_va = nc.dram_tensor("dbg_va", [128, KC * 65], f16, kind="ExternalOutput")
        dbg_pt = nc.dram_tensor("dbg_pt", [128, EB], f16, kind="ExternalOutput")
        dbg_xt = nc.dram_tensor("dbg_xt", [128, S], f16, kind="ExternalOutput")
        dbg_x2 = nc.dram_tensor("dbg_x2", [64, S], f16, kind="ExternalOutput")
        dbg_xc = nc.dram_tensor("dbg_xc", [65, QB], f32, kind="ExternalOutput")
        dbg_rd = nc.dram_tensor("dbg_rd", [1, QB], f32, kind="ExternalOutput")
        dbg_rdb = nc.dram_tensor("dbg_rdb", [64, QB], f32, kind="ExternalOutput")

    with tile.TileContext(nc) as tc:
        with (
            tc.tile_pool(name="res", bufs=1) as res,
            tc.tile_pool(name="pt", bufs=16) as pt_pool,
            tc.tile_pool(name="xc", bufs=4) as xc_pool,
            tc.tile_pool(name="ob", bufs=2) as ob_pool,
            tc.tile_pool(name="psS", bufs=2, space="PSUM") as psS,
            tc.tile_pool(name="psX", bufs=2, space="PSUM") as psX,
            tc.tile_pool(name="psP", bufs=2, space="PSUM") as psP,
        ):
            ring_state = [0]

            def ring3():
                ring_state[0] = (ring_state[0] + 1) % 3
                return (nc.sync, nc.scalar, nc.gpsimd)[ring_state[0]]

            ring2_state = [0]

            def ring2():
                # mid-kernel rings: keep the Act sequencer free for exps
                ring2_state[0] ^= 1
                return (nc.sync, nc.gpsimd)[ring2_state[0]]

            # ---- resident tiles ----
            wq_bf = res.tile([128, DCH, DP], f16, tag="wq")
            wk_bf = res.tile([128, DCH, DP], f16, tag="wk")
            wv_bf = res.tile([128, DCH, DP], f16, tag="wv")
            wo_pair = res.tile([128, D], f16, tag="wop")
            wo_h2 = res.tile([64, D], f16, tag="wo2")
            qt_bf = res.tile([128, DCH, S], f16, tag="qt")
            kt_bf = res.tile([128, DCH, S], f16, tag="kt")
            vt_bf = res.tile([128, DCH, S], f16, tag="vt")
            qT01 = res.tile([128, S], f16, tag="qT01")
            kT01 = res.tile([128, S], f16, tag="kT01")
            vT01 = res.tile([128, S], f16, tag="vT01")
            qT2 = res.tile([64, S], f16, tag="qT2")
            kT2 = res.tile([64, S], f16, tag="kT2")
            vT2 = res.tile([64, S], f16, tag="vT2")
            vaug = [
                res.tile([128, KC, 65], f16, tag=f"vaug{h}", name=f"vaug{h}")
                for h in range(HPC)
            ]
            xt01 = res.tile([128, S], f16, tag="xt01")
            xt2 = res.tile([64, S], f16, tag="xt2")
            ident = res.tile([128, 64], f16, tag="ident")

            # ---- prologue DMAs in consumption-order waves (the 3 issue
            # queues share ~330GB/s of HBM; first slabs only need k/q cols
            # 0-1023, so later columns ride behind the pipeline) ----
            nc.sync.dma_start(wq_bf[:], wqt.rearrange("p (c o) -> p c o", c=DCH))
            nc.scalar.dma_start(wk_bf[:], wkt.rearrange("p (c o) -> p c o", c=DCH))
            nc.gpsimd.dma_start(wv_bf[:], wvt.rearrange("p (c o) -> p c o", c=DCH))
            lo, hi = slice(0, EB), slice(EB, S)
            for i in range(DCH):
                ring3().dma_start(kt_bf[:, i, lo], xkt[128 * i : 128 * (i + 1), lo])
            for i in range(DCH):
                ring3().dma_start(qt_bf[:, i, lo], xqt[128 * i : 128 * (i + 1), lo])
            for i in range(DCH):
                ring3().dma_start(kt_bf[:, i, hi], xkt[128 * i : 128 * (i + 1), hi])
            for g in range(3):
                ring3().dma_start(
                    vt_bf[:, 2 * g : 2 * g + 2, lo],
                    xvt[256 * g : 256 * (g + 1), lo].rearrange(
                        "(c p) s -> p c s", p=128
                    ),
                )
            for g in range(3):
                ring3().dma_start(
                    vt_bf[:, 2 * g : 2 * g + 2, hi],
                    xvt[256 * g : 256 * (g + 1), hi].rearrange(
                        "(c p) s -> p c s", p=128
                    ),
                )
            nc.sync.dma_start(wo_pair[:], wot[0:128, :])
            nc.gpsimd.dma_start(wo_h2[:], wot[128:DP, :])
            for i in range(DCH):
                ring3().dma_start(qt_bf[:, i, hi], xqt[128 * i : 128 * (i + 1), hi])
            make_identity(nc, ident[0:64, :])
            nc.vector.tensor_copy(ident[64:128, :], ident[0:64, :])

            # ---- projection groups ----
            KINDS = {
                "q": (wq_bf, qt_bf, qT01, qT2),
                "k": (wk_bf, kt_bf, kT01, kT2),
                "v": (wv_bf, vt_bf, vT01, vT2),
            }

            def proj_group(kind, mt, j):
                """One M-tile (mt=0: heads01 pair M=128; mt=1: head2 M=64)
                of a 512-col projection block j."""
                wbf, xbf, dpair, dsingle = KINDS[kind]
                if mt == 0:
                    pp = psP.tile([128, QB], f32, tag="pp")
                    wsl = slice(0, 128)
                    dst = dpair[:, QB * j : QB * (j + 1)]
                else:
                    pp = psP.tile([64, QB], f32, tag="pp")
                    wsl = slice(128, DP)
                    dst = dsingle[:, QB * j : QB * (j + 1)]
                for i in range(DCH):
                    nc.tensor.matmul(
                        pp[:],
                        wbf[:, i, wsl],
                        xbf[:, i, QB * j : QB * (j + 1)],
                        start=(i == 0),
                        stop=(i == DCH - 1),
                    )
                if kind == "q":
                    # fold the 1/sqrt(dk) softmax scale into the Q eviction
                    nc.vector.tensor_scalar_mul(dst, pp[:], SCALE)
                else:
                    nc.vector.tensor_copy(dst, pp[:])

            def vaug_unit(h, t4):
                """4 PE transposes of head h's vT -> vaug rows + evict."""
                def emit():
                    if t4 == 0:
                        nc.vector.memset(vaug[h][:, :, 64:65], 1.0)
                    if h == 0:
                        src = lambda t: vT01[0:64, 128 * t : 128 * (t + 1)]
                        idn = ident[0:64, :]
                    elif h == 1:
                        src = lambda t: vT01[64:128, 128 * t : 128 * (t + 1)]
                        idn = ident[64:128, :]
                    else:
                        src = lambda t: vT2[:, 128 * t : 128 * (t + 1)]
                        idn = ident[0:64, :]
                    tp = psP.tile([128, 4, 64], f16, tag="pp")
                    for u in range(4):
                        nc.tensor.transpose(tp[:, u, :], src(4 * t4 + u), idn)
                    nc.vector.tensor_copy(
                        vaug[h][:, 4 * t4 : 4 * t4 + 4, 0:64], tp[:]
                    )
                return emit

            ob_tiles = {}

            def outproj_unit(j, m, tail=False):
                def emit():
                    if m == 0:
                        ob_tiles[j] = ob_pool.tile(
                            [128, DCH, QB], f16, tag="ob", name="ob"
                        )
                    op = psP.tile([128, QB], f32, tag="pp")
                    nc.tensor.matmul(
                        op[:],
                        wo_pair[:, 128 * m : 128 * (m + 1)],
                        xt01[:, QB * j : QB * (j + 1)],
                        start=True,
                        stop=False,
                    )
                    nc.tensor.matmul(
                        op[:],
                        wo_h2[:, 128 * m : 128 * (m + 1)],
                        xt2[:, QB * j : QB * (j + 1)],
                        start=False,
                        stop=True,
                    )
                    if tail and m % 2 == 0:
                        nc.scalar.copy(ob_tiles[j][:, m, :], op[:])
                    else:
                        nc.vector.tensor_copy(ob_tiles[j][:, m, :], op[:])
                    # per-chunk DMA: starts draining while later chunks evict
                    ring2().dma_start(
                        outt.rearrange("(c p) s -> p c s", p=128)[
                            :, m, QB * j : QB * (j + 1)
                        ],
                        ob_tiles[j][:, m, :],
                    )
                return emit

            # ---- slab machinery ----
            slabs = [(0, 0), (1, 0), (2, 0), (0, 1), (1, 1), (2, 1)]  # (h, e)

            def head_rows(h, tensor01, tensor2):
                if h == 0:
                    return tensor01[0:64, :]
                if h == 1:
                    return tensor01[64:128, :]
                return tensor2[:, :]

            sp_tiles = {}
            pt_tiles = {}
            xp_tiles = {}

            def emit_scores(si, t):
                h, e = slabs[si]
                kT = head_rows(h, kT01, kT2)
                qT = head_rows(h, qT01, qT2)
                sp = psS.tile([128, EB], f32, tag="sp")
                sp_tiles[(si, t)] = sp
                for half in range(2):
                    nc.tensor.matmul(
                        sp[:, QB * half : QB * (half + 1)],
                        kT[:, 128 * t : 128 * (t + 1)],
                        qT[:, EB * e + QB * half : EB * e + QB * (half + 1)],
                        start=True,
                        stop=True,
                    )

            def emit_exp(si, t):
                pt = pt_pool.tile([128, EB], f16, tag="pt")
                pt_tiles[(si, t)] = pt
                nc.scalar.activation(pt[:], sp_tiles.pop((si, t))[:], EXP)
                if _DBG and si == 0 and t == 0:
                    nc.sync.dma_start(dbg_pt[:], pt[:])

            def emit_norm_half(si, a, xp):
                """Evict one [65,512] AV accumulator and normalize into xt."""
                h, e = slabs[si]
                cols = slice(EB * e + QB * a, EB * e + QB * (a + 1))
                xca = xc_pool.tile([65, QB], f32, tag="xc")
                nc.vector.tensor_copy(xca[:], xp[:])
                # partition_broadcast reads ABSOLUTE partition 0 on HW, so the
                # den row must first move to a base-0 tile
                d0 = xc_pool.tile([1, QB], f32, tag="rd")
                nc.vector.tensor_copy(d0[:], xca[64:65, :])
                rdenb = xc_pool.tile([64, QB], f32, tag="rdb")
                nc.gpsimd.partition_broadcast(rdenb[:], d0[:])
                nc.vector.reciprocal_approx_fast(rdenb[:], rdenb[:])
                if h == 0:
                    dst = xt01[0:64, cols]
                elif h == 1:
                    dst = xt01[64:128, cols]
                else:
                    dst = xt2[:, cols]
                nc.vector.tensor_tensor(dst, xca[0:64, :], rdenb[:], MULT)
                if _DBG and si == 0 and a == 0:
                    nc.sync.dma_start(dbg_xc[:], xca[:])
                    nc.sync.dma_start(dbg_rdb[:], rdenb[:])
                normed[2 * e + a] += 1

            def emit_av(si, t):
                h, e = slabs[si]
                if si not in xp_tiles:
                    xp_tiles[si] = (
                        psX.tile([65, QB], f32, tag="xp", name="xpa"),
                        psX.tile([65, QB], f32, tag="xp", name="xpb"),
                    )
                xpa, xpb = xp_tiles[si]
                pt = pt_tiles[(si, t)]
                nc.tensor.matmul(
                    xpa[:], vaug[h][:, t, :], pt[:, 0:QB],
                    start=(t == 0), stop=(t == KC - 1),
                )
                nc.tensor.matmul(
                    xpb[:], vaug[h][:, t, :], pt[:, QB:EB],
                    start=(t == 0), stop=(t == KC - 1),
                )
                if t == KC - 1:
                    emit_norm_half(si, 0, xpa)
                    emit_norm_half(si, 1, xpb)
                    del xp_tiles[si]
                    for u in range(KC):
                        del pt_tiles[(si, u)]

            # ---- filler queue (order encodes deadlines) ----
            vaug_done = [0, 0, 0]
            normed = [0, 0, 0, 0]  # heads normed per 512-col block

            def count_vaug(h, t4):
                u = vaug_unit(h, t4)
                def emit():
                    u()
                    vaug_done[h] += 1
                return emit

            fillers = []
            fillers.append(lambda: proj_group("k", 0, 1))
            fillers.append(lambda: proj_group("v", 0, 0))
            fillers.append(lambda: proj_group("k", 0, 2))
            fillers.append(lambda: proj_group("v", 0, 1))
            fillers.append(lambda: proj_group("k", 0, 3))
            fillers.append(count_vaug(0, 0))
            fillers.append(lambda: proj_group("v", 0, 2))
            fillers.append(count_vaug(0, 1))
            fillers.append(lambda: proj_group("v", 0, 3))
            fillers.append(count_vaug(0, 2))
            fillers.append(count_vaug(0, 3))
            for t4 in range(4):
                fillers.append(count_vaug(1, t4))
            # head-2 projections (needed from slab 2 = slot 32)
            for j in range(4):
                fillers.append(lambda j=j: proj_group("k", 1, j))
            fillers.append(lambda: proj_group("q", 1, 0))
            fillers.append(lambda: proj_group("q", 1, 1))
            for j in range(4):
                fillers.append(lambda j=j: proj_group("v", 1, j))
            for t4 in range(4):
                fillers.append(count_vaug(2, t4))
            # q cols 1024-2047 (needed from slab 3 = slot 48)
            fillers.append(lambda: proj_group("q", 0, 2))
            fillers.append(lambda: proj_group("q", 0, 3))
            fillers.append(lambda: proj_group("q", 1, 2))
            fillers.append(lambda: proj_group("q", 1, 3))
            # early out-projection: reserved for slabs 4-5 where the PE
            # otherwise idles at Act pace (scores can't run ahead of exp)
            late_fillers = []
            for j in (0, 1):
                for m in range(DCH):
                    late_fillers.append(
                        (lambda j=j: normed[j] >= HPC, outproj_unit(j, m))
                    )

            # ---- prologue projections: enough for slab 0 ----
            proj_group("k", 0, 0)
            proj_group("q", 0, 0)
            proj_group("q", 0, 1)

            # ---- the 96-slot weave ----
            av_due = []   # (global_slot_emitted, si, t)
            av_ptr = [0]

            def pop_avs(s_now, budget):
                n = 0
                while n < budget and av_ptr[0] < len(av_due):
                    s_e, si, t = av_due[av_ptr[0]]
                    if s_e > s_now - 2:
                        break
                    h, _ = slabs[si]
                    if vaug_done[h] <= t // 4:
                        break
                    emit_av(si, t)
                    av_ptr[0] += 1
                    n += 1
                return n

            emit_scores(0, 0)
            for s in range(96):
                si, t = divmod(s, 16)
                emit_exp(si, t)
                av_due.append((s, si, t))
                if t < KC - 1:
                    emit_scores(si, t + 1)
                elif si < len(slabs) - 1:
                    emit_scores(si + 1, 0)
                npop = pop_avs(s, 5 if s >= 88 else 3)
                nf = 2 if npop == 0 else 1
                for _ in range(nf):
                    if not fillers:
                        break
                    head = fillers[0]
                    if isinstance(head, tuple):
                        gate, fn = head
                        if not gate():
                            break
                        fillers.pop(0)
                        fn()
                    else:
                        fillers.pop(0)()
                if s >= 48 and late_fillers and (not fillers or s % 2 == 0):
                    gate, fn = late_fillers[0]
                    if gate():
                        late_fillers.pop(0)
                        fn()

            # ---- tail: drain AVs/norms, leftover fillers, outproj j2,j3 ----
            while av_ptr[0] < len(av_due):
                pop_avs(10**9, 10**9)
            for u in fillers + late_fillers:
                u[1]() if isinstance(u, tuple) else u()
            for j in (2, 3):
                for m in range(DCH):
                    outproj_unit(j, m, tail=True)()
            if _DBG:
                nc.sync.dma_start(dbg_qT[:], qT01[:])
                nc.sync.dma_start(dbg_kT[:], kT01[:])
                nc.sync.dma_start(
                    dbg_va[:], vaug[0].rearrange("p a b -> p (a b)")
                )
                nc.sync.dma_start(dbg_xt[:], xt01[:])
                nc.sync.dma_start(dbg_x2[:], xt2[:])

    nc.compile()
    return nc


def _get_nc():
    if "nc" not in _compiled:
        _compiled["nc"] = _build()
    return _compiled["nc"]


def _pack_w(w, cols):
    # [128, DCH*DP]: partition p, free = (chunk c, outdim o) of w[cols].T
    wt = w[cols, :].T.astype(np.float16)          # [D, DP]
    return np.ascontiguousarray(
        wt.reshape(DCH, 128, DP).transpose(1, 0, 2).reshape(128, DCH * DP)
    )


def _shard(q, k, v, w_q, w_k, w_v, w_o):
    f16 = np.float16
    in_maps = []
    for c in range(NCORES):
        b, g = divmod(c, NCORES // B)
        cols = slice(DP * g, DP * (g + 1))
        in_maps.append(
            {
                "xqt": np.ascontiguousarray(q[b].T.astype(f16)),
                "xkt": np.ascontiguousarray(k[b].T.astype(f16)),
                "xvt": np.ascontiguousarray(v[b].T.astype(f16)),
                "wqt": _pack_w(w_q, cols),
                "wkt": _pack_w(w_k, cols),
                "wvt": _pack_w(w_v, cols),
                "wot": np.ascontiguousarray(w_o[:, cols].T.astype(f16)),
            }
        )
    return in_maps


def kernel(q, k, v, w_q, w_k, w_v, w_o, _trace=False):
    from concourse.bass_utils import run_bass_kernel_spmd

    q = np.asarray(q, np.float32)
    k = np.asarray(k, np.float32)
    v = np.asarray(v, np.float32)
    w_q = np.asarray(w_q, np.float32)
    w_k = np.asarray(w_k, np.float32)
    w_v = np.asarray(w_v, np.float32)
    w_o = np.asarray(w_o, np.float32)

    nc = _get_nc()
    in_maps = _shard(q, k, v, w_q, w_k, w_v, w_o)
    res = run_bass_kernel_spmd(nc, in_maps, list(range(NCORES)), trace=_trace)
    out = np.zeros((B, S, D), np.float32)
    for c in range(NCORES):
        b = c // (NCORES // B)
        out[b] += res.results[c]["outt"].T.astype(np.float32)
    if _trace:
        return out, res
    return out
